# revision 20
# baseline (speedup 1.0000x reference)
"""Trainium2 Bass kernel for nn_C2VQ_Recon_Net (class-conditional VQ recon net).

Strategy (8 NeuronCores, SPMD, zero collectives):
  - Host: order samples by class (class order chosen so each contiguous
    64-sample block spans <= S classes, S usually 2), one block per core.
  - Each core: conv encoder (fp32, float32r matmuls) -> class-sliced VQ
    distance GEMM (fp32) -> argmax via max_index -> dynamic-DMA gather of
    codebook rows -> conv decoder (bf16) -> sigmoid.
  - Host: reassemble outputs (the reference's stable-argsort permutation is
    applied on the host via output row placement).

Self-contained: hardcodes all shapes; reads nothing from disk.
"""
import numpy as np

from concourse import bass, mybir
from concourse import bacc
from concourse import tile
from concourse.bass_utils import run_bass_kernel_spmd

K, NCLS, CD, SP = 2048, 8, 256, 4
D = CD * SP * SP       # 4096
NTOT = 512
NCORES = 8
F32 = mybir.dt.float32
F32R = mybir.dt.float32r
BF16 = mybir.dt.bfloat16
U32 = mybir.dt.uint32
AF = mybir.ActivationFunctionType
ALU = mybir.AluOpType
NEG = -1.0e30


# ----------------------------------------------------------------------------
# host helpers
# ----------------------------------------------------------------------------

def _class_order(counts):
    """Order the 8 classes so that cutting the concatenated (ordered) samples
    into 8 blocks of 64 gives each block at most 2 distinct classes.
    Brute force over a limited number of permutations; fall back to identity."""
    import itertools
    best, best_s = None, 99
    for i, perm in enumerate(itertools.permutations(range(NCLS))):
        if i > 45000:
            break
        cs = np.cumsum([counts[p] for p in perm])
        # interior class boundaries strictly inside a block -> splits
        nspan = 1
        maxspan = 1
        for j in range(NCORES):
            lo, hi = 64 * j, 64 * j + 64
            nb = sum(1 for b in cs[:-1] if lo < b < hi)
            maxspan = max(maxspan, nb + 1)
        if maxspan < best_s:
            best_s, best = maxspan, perm
        if best_s == 1:
            break
        if best_s == 2 and i > 5000:
            break
    return list(best), best_s


def _up_tab(H):
    Ho = 2 * H
    pos = np.arange(Ho) * (H - 1) / (Ho - 1)
    x0 = np.floor(pos).astype(np.int64)
    w = (pos - x0).astype(np.float32)
    sel = x0 >= H - 1
    x0[sel] = H - 2
    w[sel] = 1.0
    # exact-zero endpoints
    w[np.abs(w) < 1e-9] = 0.0
    return x0, w


def _pack_w(W):
    """W [co=256, ci=256, 3, 3] -> [128(ci_in), 2(cit)*9(tap)*2(cot)*128(co)]"""
    Wr = W.reshape(2, 128, 2, 128, 3, 3)           # cot, co_in, cit, ci_in, dy, dx
    A = np.transpose(Wr, (3, 2, 4, 5, 0, 1))       # ci_in, cit, dy, dx, cot, co_in
    return np.ascontiguousarray(A.reshape(128, 2 * 9 * 2 * 128), np.float32)


# ----------------------------------------------------------------------------
# device graph
# ----------------------------------------------------------------------------

def build_graph(S, dec_bf16=True, dbg=False, no_gather=False, no_colpack=False):
    """Build the SPMD per-core graph. Per-core sample count fixed at 64."""
    NS = 64            # samples per core
    GE = 16            # encoder group
    GD = 8             # decoder group
    NGE = NS // GE
    NGD = NS // GD
    S2K = S * K

    DT = BF16 if dec_bf16 else F32

    nc = bacc.Bacc(None, target_bir_lowering=False, debug=False)

    # ---- dram parameters (inputs) ----
    xpatch_d = nc.declare_dram_parameter("xpatch", [9, NS, 196], F32, isOutput=False)
    w1_d = nc.declare_dram_parameter("w1", [9, 2, 128], F32, isOutput=False)
    w2_d = nc.declare_dram_parameter("w2", [128, 4608], F32, isOutput=False)
    w3_d = nc.declare_dram_parameter("w3", [128, 4608], F32, isOutput=False)
    wd_d = [nc.declare_dram_parameter(f"wd{i}", [128, 4608], F32, isOutput=False)
            for i in range(3)]
    wo_d = nc.declare_dram_parameter("wo", [128, 18], F32, isOutput=False)
    bias_d = nc.declare_dram_parameter("biasv", [128, 12], F32, isOutput=False)
    ep_d = nc.declare_dram_parameter("epermT", [32, 128, S2K], F32, isOutput=False)
    mask_d = nc.declare_dram_parameter("maskadd", [64, S2K], F32, isOutput=False)
    e4_d = nc.declare_dram_parameter("e4", [S2K, 2, 128, 16], F32, isOutput=False)
    # outputs
    xt_d = nc.declare_dram_parameter("xt", [NS, 784], F32, isOutput=True)
    if dbg:
        dbg_d = {nm: nc.declare_dram_parameter(f"dbg_{nm}", shp, F32, isOutput=True)
                 for nm, shp in [("qb", [128, 8, 4, 4]), ("up1", [128, 8, 10, 10]),
                                 ("h4", [128, 8, 8, 8]), ("up2", [128, 8, 16, 16]),
                                 ("h5", [128, 8, 14, 14]), ("up3", [128, 8, 30, 30]),
                                 ("h6", [128, 8, 30, 30])]}
    ze_d = nc.declare_dram_parameter("ze", [NS, 2, 128, 16], F32, isOutput=True)
    zq_d = nc.declare_dram_parameter("zq", [NS, 2, 128, 16], F32, isOutput=True)

    ut1 = _up_tab(4)
    ut2 = _up_tab(8)
    ut3 = _up_tab(14)

    from contextlib import ExitStack
    with tile.TileContext(nc) as tc:
        with (
            tc.tile_pool(name="const", bufs=1) as constp,
            tc.tile_pool(name="wpool", bufs=3) as wpool,
            tc.tile_pool(name="zhq", bufs=1) as zhp,
            tc.tile_pool(name="ep", bufs=4) as epp,
            tc.tile_pool(name="pmm", bufs=3, space="PSUM") as pmm,
            tc.tile_pool(name="pvq", bufs=2, space="PSUM") as pvq,
            tc.tile_pool(name="poc", bufs=2, space="PSUM") as poc,
        ):
            # ---------------- constants / weights ----------------
            w1 = constp.tile([9, 2, 128], F32)
            nc.sync.dma_start(w1[:], w1_d[:])
            biasv = constp.tile([128, 12], F32)
            nc.sync.dma_start(biasv[:], bias_d[:])
            wo = constp.tile([128, 18], DT)
            nc.gpsimd.dma_start(wo[:], wo_d[:])

            w2 = wpool.tile([128, 4608], F32, tag="w")
            nc.sync.dma_start(w2[:], w2_d[:])
            w3 = wpool.tile([128, 4608], F32, tag="w")
            nc.sync.dma_start(w3[:], w3_d[:])

            def wsl(w, cit, tap, cot):
                base = ((cit * 9 + tap) * 2 + cot) * 128
                return w[:, base:base + 128]

            # persistent z_e (conv3 out), layout [co_in, n, ij] per co-half
            zh = [zhp.tile([128, NS, 16], F32, tag=f"zh{h}", name=f"zh{h}") for h in range(2)]

            # ---------------- encoder ----------------
            enc_stack = ExitStack()
            encp = enc_stack.enter_context(tc.tile_pool(name="enc", bufs=1))
            xpp = enc_stack.enter_context(tc.tile_pool(name="xp", bufs=2))
            for g in range(NGE):
                n0 = g * GE
                xpatch = xpp.tile([9, GE, 196], F32, tag="xp", name=f"xp{g}")
                nc.sync.dma_start(xpatch[:], xpatch_d[:, n0:n0 + GE, :])
                h1p = [encp.tile([128, GE, 16, 16], F32, tag=f"h1p{t}", name=f"h1p{t}_{g}") for t in range(2)]
                h2p = [encp.tile([128, GE, 9, 9], F32, tag=f"h2p{t}", name=f"h2p{t}_{g}") for t in range(2)]
                for t in range(2):
                    # zero the 1-wide borders
                    nc.gpsimd.memset(h1p[t][:, :, 0, :], 0.0)
                    nc.gpsimd.memset(h1p[t][:, :, 15, :], 0.0)
                    nc.gpsimd.memset(h1p[t][:, :, 1:15, 0], 0.0)
                    nc.gpsimd.memset(h1p[t][:, :, 1:15, 15], 0.0)
                    nc.gpsimd.memset(h2p[t][:, :, 0, :], 0.0)
                    nc.gpsimd.memset(h2p[t][:, :, 8, :], 0.0)
                    nc.gpsimd.memset(h2p[t][:, :, 1:8, 0], 0.0)
                    nc.gpsimd.memset(h2p[t][:, :, 1:8, 8], 0.0)

                # conv1: K=9, chunks of 2 samples (392 cols)
                for cot in range(2):
                    for cs in range(0, GE, 2):
                        ps = pmm.tile([128, 512], F32, tag="mm")
                        rhs = xpatch[:, cs:cs + 2, :]
                        nc.tensor.matmul(ps[:, :392],
                                         w1[:, cot, :],
                                         rhs, start=True, stop=True)
                        nc.scalar.activation(
                            h1p[cot][:, cs:cs + 2, 1:15, 1:15],
                            ps[:, :392], AF.Relu, bias=biasv[:, 0 + cot:1 + cot])

                # conv2: 16x16(pad) -> 7x7, chunks of 8 samples (392 cols)
                for cot in range(2):
                    for cs in range(0, GE, 8):
                        ps = pmm.tile([128, 512], F32, tag="mm")
                        kk = 0
                        for tap in range(9):
                            dy, dx = tap // 3, tap % 3
                            for cit in range(2):
                                rhs = h1p[cit][:, cs:cs + 8, dy:dy + 14:2, dx:dx + 14:2]
                                nc.tensor.matmul(
                                    ps[:, :392], wsl(w2, cit, tap, cot),
                                    rhs,
                                    start=(kk == 0), stop=(kk == 17))
                                kk += 1
                        nc.scalar.activation(
                            h2p[cot][:, cs:cs + 8, 1:8, 1:8],
                            ps[:, :392], AF.Relu, bias=biasv[:, 2 + cot:3 + cot])

                # conv3: 9x9(pad) -> 4x4, one chunk (GE*16 = 256 cols), no relu
                for cot in range(2):
                    ps = pmm.tile([128, 512], F32, tag="mm")
                    kk = 0
                    for tap in range(9):
                        dy, dx = tap // 3, tap % 3
                        for cit in range(2):
                            rhs = h2p[cit][:, :, dy:dy + 7:2, dx:dx + 7:2]
                            nc.tensor.matmul(
                                ps[:, :GE * 16], wsl(w3, cit, tap, cot),
                                rhs,
                                start=(kk == 0), stop=(kk == 17))
                            kk += 1
                    nc.scalar.activation(
                        zh[cot][:, n0:n0 + GE, :],
                        ps[:, :GE * 16], AF.Identity, bias=biasv[:, 4 + cot:5 + cot])

            # z_e output
            for h in range(2):
                nc.sync.dma_start(ze_d[:, h, :, :].transpose([1, 0, 2]), zh[h][:])

            enc_stack.close()

            # ---------------- VQ distance GEMM ----------------
            # score[n, code] = 2*z.e - |e|^2 + mask ; argmax == argmin d2
            vq_stack = ExitStack()
            vqp = vq_stack.enter_context(tc.tile_pool(name="vq", bufs=1))
            score = vqp.tile([64, S2K], F32)

            NCH = S2K // 512
            for ch in range(NCH):
                c0 = ch * 512
                maskt = vqp.tile([64, 512], F32, tag="mk", name=f"mk{ch}", bufs=2)
                nc.sync.dma_start(maskt[:], mask_d[:, c0:c0 + 512])
                ps = pvq.tile([64, 512], F32, tag="vq")
                for t in range(32):
                    ij, hf = t // 2, t % 2
                    ept = epp.tile([128, 512], F32, tag="ep", name=f"ep{ch}_{t}")
                    nc.sync.dma_start(ept[:], ep_d[t, :, c0:c0 + 512])
                    nc.tensor.matmul(ps, zh[hf][:, :, ij],
                                     ept[:],
                                     start=(t == 0), stop=(t == 31))
                nc.vector.tensor_tensor(score[:, c0:c0 + 512], ps,
                                        maskt[:], ALU.add)

            mx = zhp.tile([64, 8], F32)
            mi = zhp.tile([64, 8], U32)
            nc.vector.max(mx[:], score[:])
            nc.vector.max_index(mi[:], mx[:], score[:])

            # ---------------- gather E[idx] ----------------
            q = [zhp.tile([128, NS, 16], F32, tag=f"q{h}", name=f"q{h}") for h in range(2)]
            for n in range(NS):
                if no_gather:
                    for h in range(2):
                        nc.sync.dma_start(q[h][:, n, :], e4_d[n, h, :, :])
                else:
                    val = nc.values_load(mi[n:n + 1, 0:1],
                                         engines=(mybir.EngineType.Pool,),
                                         min_val=0, max_val=S2K - 1,
                                         skip_runtime_bounds_check=True)
                    for h in range(2):
                        nc.gpsimd.dma_start(q[h][:, n, :],
                                            e4_d[bass.ds(val, 1), h, :, :])
            for h in range(2):
                nc.sync.dma_start(zq_d[:, h, :, :].transpose([1, 0, 2]), q[h][:])

            vq_stack.close()

            # ---------------- decoder ----------------
            dec_stack = ExitStack()
            decp = dec_stack.enter_context(tc.tile_pool(name="dec", bufs=1))
            decp2 = dec_stack.enter_context(tc.tile_pool(name="dec2", bufs=1))
            wd = [wpool.tile([128, 4608], DT, tag="w", name=f"wdt{i}") for i in range(3)]
            for i in range(3):
                nc.gpsimd.dma_start(wd[i][:], wd_d[i][:])

            def upsample(src, dst, tab, H, W, pad, engs):
                """src [128, GD, H, W] -> dst interior [128, GD, 2H, 2W] with
                optional 1-border (dst preallocated; border zeroed by caller).
                engs: (e0, e1) engines for x / y passes."""
                x0t, wxt = tab
                Ho, Wo = 2 * H, 2 * W
                e0, e1 = engs
                tmp = decp2.tile([128, GD, H, Wo], DT, tag=f"tmp", name="tmp", bufs=2)
                dx = decp2.tile([128, GD, H, W - 1], DT, tag=f"dd", name="dx", bufs=2)
                e0.tensor_tensor(dx[:], src[:, :, :, 1:], src[:, :, :, :-1],
                                 ALU.subtract)
                for ox in range(Wo):
                    x0, w = int(x0t[ox]), float(wxt[ox])
                    if w == 0.0:
                        e0.tensor_copy(tmp[:, :, :, ox], src[:, :, :, x0])
                    else:
                        e0.tensor_scalar_mul(tmp[:, :, :, ox], dx[:, :, :, x0], w)
                        e0.tensor_tensor(tmp[:, :, :, ox], tmp[:, :, :, ox],
                                         src[:, :, :, x0], ALU.add)
                dy = decp2.tile([128, GD, H - 1, Wo], DT, tag=f"dd2", name="dy", bufs=2)
                e1.tensor_tensor(dy[:], tmp[:, :, 1:, :], tmp[:, :, :-1, :],
                                 ALU.subtract)
                p = 1 if pad else 0
                for oy in range(Ho):
                    y0, w = int(x0t[oy]), float(wxt[oy])
                    out = dst[:, :, oy + p, p:p + Wo]
                    if w == 0.0:
                        e1.tensor_copy(out, tmp[:, :, y0, :])
                    else:
                        e1.tensor_scalar_mul(out, dy[:, :, y0, :], w)
                        e1.tensor_tensor(out, out, tmp[:, :, y0, :], ALU.add)

            def zero_border(t, Hp, Wp):
                nc.gpsimd.memset(t[:, :, 0, :], 0.0)
                nc.gpsimd.memset(t[:, :, Hp - 1, :], 0.0)
                nc.gpsimd.memset(t[:, :, 1:Hp - 1, 0], 0.0)
                nc.gpsimd.memset(t[:, :, 1:Hp - 1, Wp - 1], 0.0)

            def dconv(win, src, cnt_list, outs):
                """3x3 stride-1 valid conv on (padded) src tiles.
                src: [2][128, GD, HS, WS]; cnt_list: (ns, ncnt, r0, Ho, Wo).
                outs(cot, chunk_idx, ps, ncols) -> epilogue."""
                for cot in range(2):
                    for ci_, (ns, ncnt, r0, Ho, Wo) in enumerate(cnt_list):
                        ncols = ncnt * Ho * Wo
                        ps = pmm.tile([128, 512], F32, tag="mm")
                        kk = 0
                        for tap in range(9):
                            dyy, dxx = tap // 3, tap % 3
                            for cit in range(2):
                                rhs = src[cit][:, ns:ns + ncnt,
                                               r0 + dyy:r0 + dyy + Ho,
                                               dxx:dxx + Wo]
                                nc.tensor.matmul(
                                    ps[:, :ncols], wsl(win, cit, tap, cot).bitcast(DT),
                                    rhs.bitcast(DT),
                                    start=(kk == 0), stop=(kk == 17))
                                kk += 1
                        outs(cot, ci_, ps, ncols)

            for g in range(NGD):
                n0 = g * GD
                qb = [decp.tile([128, GD, 4, 4], DT, tag=f"qb{h}", name=f"qb{h}_{g}") for h in range(2)]
                for h in range(2):
                    nc.vector.tensor_copy(
                        qb[h][:], q[h][:, n0:n0 + GD, :])

                up1 = [decp.tile([128, GD, 10, 10], DT, tag=f"up1{h}", name=f"up1{h}_{g}") for h in range(2)]
                for h in range(2):
                    zero_border(up1[h], 10, 10)
                    upsample(qb[h], up1[h], ut1, 4, 4, True,
                             (nc.vector, nc.gpsimd) if h == 0 else (nc.gpsimd, nc.vector))

                h4 = [decp.tile([128, GD, 8, 8], DT, tag=f"h4{h}", name=f"h4{h}_{g}") for h in range(2)]

                def out_d0(cot, ci_, ps, ncols):
                    nc.scalar.activation(h4[cot][:],
                                         ps[:, :ncols], AF.Relu,
                                         bias=biasv[:, 6 + cot:7 + cot])
                dconv(wd[0], up1, [(0, GD, 0, 8, 8)], out_d0)

                up2 = [decp.tile([128, GD, 16, 16], DT, tag=f"up2{h}", name=f"up2{h}_{g}") for h in range(2)]
                for h in range(2):
                    upsample(h4[h], up2[h], ut2, 8, 8, False,
                             (nc.vector, nc.gpsimd) if h == 0 else (nc.gpsimd, nc.vector))

                h5 = [decp.tile([128, GD, 14, 14], DT, tag=f"h5{h}", name=f"h5{h}_{g}") for h in range(2)]

                def out_d1(cot, ci_, ps, ncols):
                    ns = ci_ * 2
                    nc.scalar.activation(
                        h5[cot][:, ns:ns + 2, :, :],
                        ps[:, :ncols], AF.Relu, bias=biasv[:, 8 + cot:9 + cot])
                dconv(wd[1], up2, [(i * 2, 2, 0, 14, 14) for i in range(GD // 2)],
                      out_d1)

                up3 = [decp.tile([128, GD, 30, 30], DT, tag=f"up3{h}", name=f"up3{h}_{g}") for h in range(2)]
                for h in range(2):
                    zero_border(up3[h], 30, 30)
                    upsample(h5[h], up3[h], ut3, 14, 14, True,
                             (nc.vector, nc.gpsimd) if h == 0 else (nc.gpsimd, nc.vector))

                h6 = [decp.tile([128, GD, 30, 30], DT, tag=f"h6{h}", name=f"h6{h}_{g}") for h in range(2)]
                for h in range(2):
                    zero_border(h6[h], 30, 30)

                def out_d2(cot, ci_, ps, ncols):
                    n, r = ci_ // 2, (ci_ % 2) * 14
                    nc.scalar.activation(
                        h6[cot][:, n, 1 + r:15 + r, 1:29],
                        ps[:, :ncols], AF.Relu, bias=biasv[:, 10 + cot:11 + cot])
                dconv(wd[2], up3, [(n, 1, r, 14, 28) for n in range(GD)
                                   for r in (0, 14)], out_d2)

                # output conv: co=1, col-packed 4-wide via tile_position.
                # 16 chunks per group: (sample, half-rows); 4 chunks share a
                # psum tile on col-groups 0/32/64/96.
                chunks = [(n, r) for n in range(GD) for r in (0, 14)]
                for s4 in range(0, len(chunks), 4):
                    ps = poc.tile([128, 512], F32, tag="oc")
                    pslist = ([poc.tile([128, 512], F32, tag="oc",
                                        name=f"oc{g}_{s4}_{j}") for j in range(4)]
                              if no_colpack else None)
                    for j in range(4):
                        n, r = chunks[s4 + j]
                        kk = 0
                        for tap in range(9):
                            dyy, dxx = tap // 3, tap % 3
                            for cit in range(2):
                                rhs = h6[cit][:, n, r + dyy:r + dyy + 14,
                                              dxx:dxx + 28]
                                if no_colpack:
                                    nc.tensor.matmul(
                                        pslist[j][0:1, :392],
                                        wo[:, cit * 9 + tap:cit * 9 + tap + 1].bitcast(DT),
                                        rhs.bitcast(DT),
                                        start=(kk == 0), stop=(kk == 17))
                                else:
                                    nc.tensor.matmul(
                                        ps[32 * j:32 * j + 1, :392],
                                        wo[:, cit * 9 + tap:cit * 9 + tap + 1].bitcast(DT),
                                        rhs.bitcast(DT),
                                        start=(kk == 0), stop=(kk == 17),
                                        tile_position=(0, 32 * j))
                                kk += 1
                    xt_sb = decp.tile([128, 392], F32, tag="xt", name=f"xt{g}_{s4}",
                                      bufs=4)
                    if no_colpack:
                        xts = [decp.tile([128, 392], F32, tag="xt",
                                         name=f"xts{g}_{s4}_{j}", bufs=4)
                               for j in range(4)]
                    for j in range(4):
                        n, r = chunks[s4 + j]
                        src_ps = pslist[j][0:1, :392] if no_colpack                             else ps[32 * j:32 * j + 1, :392]
                        dst_sb = xt_sb[0:1, :] if no_colpack                             else xt_sb[32 * j:32 * j + 1, :]
                        nc.scalar.activation(dst_sb, src_ps, AF.Sigmoid,
                                             bias=BO_IMM[0])
                        nc.sync.dma_start(
                            xt_d[n0 + n:n0 + n + 1, 28 * r:28 * r + 392],
                            dst_sb)
                if dbg and g == 0:
                    for nm, t in [("qb", qb[0]), ("up1", up1[0]), ("h4", h4[0]),
                                  ("up2", up2[0]), ("h5", h5[0]), ("up3", up3[0]),
                                  ("h6", h6[0])]:
                        nc.gpsimd.dma_start(dbg_d[nm][:], t[:])
            dec_stack.close()

    nc.compile()
    return nc


BO_IMM = [0.0]   # set by kernel() before build (bo is a runtime input scalar)


# ----------------------------------------------------------------------------
# kernel entry
# ----------------------------------------------------------------------------

def prepare(inputs):
    inputs = {k: np.ascontiguousarray(np.asarray(v)) for k, v in inputs.items()}
    x = inputs['x'].astype(np.float32)           # [512, 1, 28, 28]
    c = inputs['c'].astype(np.int64)             # [512]
    E = inputs['E'].astype(np.float32)           # [16384, 4096]
    BO_IMM[0] = float(np.asarray(inputs['bo']).reshape(-1)[0])

    counts = np.bincount(c, minlength=NCLS)
    order, S = _class_order(counts)
    S = max(S, 1)

    # sample ids in shard order: classes in `order`, stable within class
    ids_by_class = [np.nonzero(c == k)[0] for k in range(NCLS)]
    shard_ids = np.concatenate([ids_by_class[k] for k in order])
    # reference output permutation (stable sort by class ascending)
    perm = np.argsort(c, kind='stable')
    # row of each sample in the permuted outputs
    pos_in_perm = np.empty(NTOT, np.int64)
    pos_in_perm[perm] = np.arange(NTOT)

    # per-core classes + padded slice lists
    blocks = [shard_ids[64 * j:64 * j + 64] for j in range(NCORES)]
    slice_lists = []
    for blk in blocks:
        cls = list(dict.fromkeys(c[blk].tolist()))   # preserves order
        while len(cls) < S:
            cls.append(cls[-1])
        assert len(cls) == S, f"block spans {len(cls)} > S={S} classes"
        slice_lists.append(cls)

    # ---- weights (shared across cores) ----
    w1 = np.ascontiguousarray(
        inputs['We0'].reshape(2, 128, 3, 3).transpose(2, 3, 0, 1).reshape(9, 2, 128),
        np.float32)
    w2 = _pack_w(inputs['We1'])
    w3 = _pack_w(inputs['We2'])
    wd = [_pack_w(inputs[f'Wd{i}']) for i in range(3)]
    wo = np.ascontiguousarray(
        inputs['Wo'].reshape(2, 128, 3, 3).transpose(1, 0, 2, 3).reshape(128, 18),
        np.float32)
    biasv = np.zeros((128, 12), np.float32)
    for L, bn in enumerate(['be0', 'be1', 'be2', 'bd0', 'bd1', 'bd2']):
        b = inputs[bn].astype(np.float32)
        biasv[:, 2 * L] = b[:128]
        biasv[:, 2 * L + 1] = b[128:]

    # ---- per-core tensors ----
    in_maps = []
    for j in range(NCORES):
        blk = blocks[j]
        cls = slice_lists[j]
        xb = x[blk, 0]                                   # [64, 28, 28]
        xp = np.pad(xb, ((0, 0), (1, 1), (1, 1)))
        patches = np.empty((9, 64, 196), np.float32)
        for t in range(9):
            dy, dx = t // 3, t % 3
            patches[t] = xp[:, dy:dy + 28:2, dx:dx + 28:2].reshape(64, 196)

        E_cat = np.concatenate([E[k * K:(k + 1) * K] for k in cls], 0)  # [S2K, 4096]
        arr = (2.0 * E_cat).reshape(S * K, 2, 128, 16)
        epermT = np.ascontiguousarray(
            arr.transpose(3, 1, 2, 0).reshape(32, 128, S * K), np.float32)
        esqneg = -(E_cat ** 2).sum(1, dtype=np.float32)
        maskadd = np.full((64, S * K), NEG, np.float32)
        for i, sid in enumerate(blk):
            s = cls.index(int(c[sid]))
            maskadd[i, s * K:(s + 1) * K] = esqneg[s * K:(s + 1) * K]
        e4 = np.ascontiguousarray(E_cat.reshape(S * K, 2, 128, 16), np.float32)

        in_maps.append(dict(
            xpatch=patches, w1=w1, w2=w2, w3=w3,
            wd0=wd[0], wd1=wd[1], wd2=wd[2], wo=wo, biasv=biasv,
            epermT=epermT, maskadd=maskadd, e4=e4,
        ))

    return in_maps, blocks, pos_in_perm, S


def assemble(results, blocks, pos_in_perm):
    # ---- reassemble ----
    x_tilde = np.empty((NTOT, 1, 28, 28), np.float32)
    z_e = np.empty((NTOT, D, 1, 1), np.float32)
    z_q = np.empty((NTOT, D, 1, 1), np.float32)
    for j in range(len(blocks)):
        blk = blocks[j]
        r = results[j]
        xt = r['xt'].reshape(64, 28, 28)
        ze = r['ze'].reshape(64, D)       # [n, (half,co,ij)] == [n, d]
        zq = r['zq'].reshape(64, D)
        for i, sid in enumerate(blk):
            z_e[sid, :, 0, 0] = ze[i]
            p = pos_in_perm[sid]
            x_tilde[p, 0] = xt[i]
            z_q[p, :, 0, 0] = zq[i]
    return (x_tilde, z_e, z_q)


def kernel(**inputs):
    in_maps, blocks, pos_in_perm, S = prepare(inputs)
    nc = build_graph(S, dec_bf16=True)
    res = run_bass_kernel_spmd(nc, in_maps, core_ids=list(range(NCORES)))
    return assemble(res.results, blocks, pos_in_perm)


# revision 22
# speedup vs baseline: 1.5923x; 1.5923x over previous
"""Trainium2 Bass kernel for nn_C2VQ_Recon_Net (class-conditional VQ recon net).

Strategy (8 NeuronCores, SPMD, zero collectives):
  - Host: order samples by class (class order chosen so each contiguous
    64-sample block spans <= S classes, S usually 2), one block per core.
  - Each core: conv encoder (fp32, float32r matmuls) -> class-sliced VQ
    distance GEMM (fp32) -> argmax via max_index -> dynamic-DMA gather of
    codebook rows -> conv decoder (bf16) -> sigmoid.
  - Host: reassemble outputs (the reference's stable-argsort permutation is
    applied on the host via output row placement).

Self-contained: hardcodes all shapes; reads nothing from disk.
"""
import numpy as np

from concourse import bass, mybir
from concourse import bacc
from concourse import tile
from concourse.bass_utils import run_bass_kernel_spmd

K, NCLS, CD, SP = 2048, 8, 256, 4
D = CD * SP * SP       # 4096
NTOT = 512
NCORES = 8
F32 = mybir.dt.float32
F32R = mybir.dt.float32r
BF16 = mybir.dt.bfloat16
U32 = mybir.dt.uint32
AF = mybir.ActivationFunctionType
ALU = mybir.AluOpType
NEG = -1.0e30


# ----------------------------------------------------------------------------
# host helpers
# ----------------------------------------------------------------------------

def _class_order(counts):
    """Order the 8 classes so that cutting the concatenated (ordered) samples
    into 8 blocks of 64 gives each block at most 2 distinct classes.
    Brute force over a limited number of permutations; fall back to identity."""
    import itertools
    best, best_s = None, 99
    for i, perm in enumerate(itertools.permutations(range(NCLS))):
        if i > 45000:
            break
        cs = np.cumsum([counts[p] for p in perm])
        # interior class boundaries strictly inside a block -> splits
        nspan = 1
        maxspan = 1
        for j in range(NCORES):
            lo, hi = 64 * j, 64 * j + 64
            nb = sum(1 for b in cs[:-1] if lo < b < hi)
            maxspan = max(maxspan, nb + 1)
        if maxspan < best_s:
            best_s, best = maxspan, perm
        if best_s == 1:
            break
        if best_s == 2 and i > 5000:
            break
    return list(best), best_s


def _up_tab(H):
    Ho = 2 * H
    pos = np.arange(Ho) * (H - 1) / (Ho - 1)
    x0 = np.floor(pos).astype(np.int64)
    w = (pos - x0).astype(np.float32)
    sel = x0 >= H - 1
    x0[sel] = H - 2
    w[sel] = 1.0
    # exact-zero endpoints
    w[np.abs(w) < 1e-9] = 0.0
    return x0, w


def _pack_w(W):
    """W [co=256, ci=256, 3, 3] -> [128(ci_in), 2(cit)*9(tap)*2(cot)*128(co)]"""
    Wr = W.reshape(2, 128, 2, 128, 3, 3)           # cot, co_in, cit, ci_in, dy, dx
    A = np.transpose(Wr, (3, 2, 4, 5, 0, 1))       # ci_in, cit, dy, dx, cot, co_in
    return np.ascontiguousarray(A.reshape(128, 2 * 9 * 2 * 128), np.float32)


# ----------------------------------------------------------------------------
# device graph
# ----------------------------------------------------------------------------

def build_graph(S, dec_bf16=True, dbg=False, no_gather=False, no_colpack=False):
    """Build the SPMD per-core graph. Per-core sample count fixed at 64."""
    NS = 64            # samples per core
    GE = 16            # encoder group
    GD = 8             # decoder group
    NGE = NS // GE
    NGD = NS // GD
    S2K = S * K

    DT = BF16 if dec_bf16 else F32

    nc = bacc.Bacc(None, target_bir_lowering=False, debug=False)

    # ---- dram parameters (inputs) ----
    xpatch_d = nc.declare_dram_parameter("xpatch", [9, NS, 196], F32, isOutput=False)
    w1_d = nc.declare_dram_parameter("w1", [9, 2, 128], F32, isOutput=False)
    w2_d = nc.declare_dram_parameter("w2", [128, 4608], F32, isOutput=False)
    w3_d = nc.declare_dram_parameter("w3", [128, 4608], F32, isOutput=False)
    wd_d = [nc.declare_dram_parameter(f"wd{i}", [128, 4608], F32, isOutput=False)
            for i in range(3)]
    wo_d = nc.declare_dram_parameter("wo", [128, 18], F32, isOutput=False)
    bias_d = nc.declare_dram_parameter("biasv", [128, 12], F32, isOutput=False)
    ep_d = nc.declare_dram_parameter("epermT", [32, 128, S2K], F32, isOutput=False)
    mask_d = nc.declare_dram_parameter("maskadd", [64, S2K], F32, isOutput=False)
    e4_d = nc.declare_dram_parameter("e4", [S2K, 2, 128, 16], F32, isOutput=False)
    # outputs
    xt_d = nc.declare_dram_parameter("xt", [NS, 784], F32, isOutput=True)
    if dbg:
        dbg_d = {nm: nc.declare_dram_parameter(f"dbg_{nm}", shp, F32, isOutput=True)
                 for nm, shp in [("qb", [128, 8, 4, 4]), ("up1", [128, 8, 10, 10]),
                                 ("h4", [128, 8, 8, 8]), ("up2", [128, 8, 16, 16]),
                                 ("h5", [128, 8, 14, 14]), ("up3", [128, 8, 30, 30]),
                                 ("h6", [128, 8, 30, 30])]}
    ze_d = nc.declare_dram_parameter("ze", [NS, 2, 128, 16], F32, isOutput=True)
    zq_d = nc.declare_dram_parameter("zq", [NS, 2, 128, 16], F32, isOutput=True)

    ut1 = _up_tab(4)
    ut2 = _up_tab(8)
    ut3 = _up_tab(14)

    from contextlib import ExitStack
    with tile.TileContext(nc) as tc:
        with (
            tc.tile_pool(name="const", bufs=1) as constp,
            tc.tile_pool(name="wpool", bufs=3) as wpool,
            tc.tile_pool(name="zhq", bufs=1) as zhp,
            tc.tile_pool(name="ep", bufs=4) as epp,
            tc.tile_pool(name="pmm", bufs=4, space="PSUM") as pmm,
            tc.tile_pool(name="pvq", bufs=2, space="PSUM") as pvq,
            tc.tile_pool(name="poc", bufs=2, space="PSUM") as poc,
        ):
            # ---------------- constants / weights ----------------
            w1 = constp.tile([9, 2, 128], F32)
            nc.sync.dma_start(w1[:], w1_d[:])
            biasv = constp.tile([128, 12], F32)
            nc.sync.dma_start(biasv[:], bias_d[:])
            wo = constp.tile([128, 18], DT)
            nc.gpsimd.dma_start(wo[:], wo_d[:])

            w2 = wpool.tile([128, 4608], F32, tag="w")
            nc.sync.dma_start(w2[:], w2_d[:])
            w3 = wpool.tile([128, 4608], F32, tag="w")
            nc.sync.dma_start(w3[:], w3_d[:])

            def wsl(w, cit, tap, cot):
                base = ((cit * 9 + tap) * 2 + cot) * 128
                return w[:, base:base + 128]

            # persistent z_e (conv3 out), layout [co_in, n, ij] per co-half
            zh = [zhp.tile([128, NS, 16], F32, tag=f"zh{h}", name=f"zh{h}") for h in range(2)]

            # ---------------- encoder ----------------
            enc_stack = ExitStack()
            encp = enc_stack.enter_context(tc.tile_pool(name="enc", bufs=1))
            xpp = enc_stack.enter_context(tc.tile_pool(name="xp", bufs=2))
            for g in range(NGE):
                n0 = g * GE
                xpatch = xpp.tile([9, GE, 196], F32, tag="xp", name=f"xp{g}")
                nc.sync.dma_start(xpatch[:], xpatch_d[:, n0:n0 + GE, :])
                h1p = [encp.tile([128, GE, 16, 16], F32, tag=f"h1p{t}", name=f"h1p{t}_{g}") for t in range(2)]
                h2p = [encp.tile([128, GE, 9, 9], F32, tag=f"h2p{t}", name=f"h2p{t}_{g}") for t in range(2)]
                for t in range(2):
                    # zero the 1-wide borders
                    nc.gpsimd.memset(h1p[t][:, :, 0, :], 0.0)
                    nc.gpsimd.memset(h1p[t][:, :, 15, :], 0.0)
                    nc.gpsimd.memset(h1p[t][:, :, 1:15, 0], 0.0)
                    nc.gpsimd.memset(h1p[t][:, :, 1:15, 15], 0.0)
                    nc.gpsimd.memset(h2p[t][:, :, 0, :], 0.0)
                    nc.gpsimd.memset(h2p[t][:, :, 8, :], 0.0)
                    nc.gpsimd.memset(h2p[t][:, :, 1:8, 0], 0.0)
                    nc.gpsimd.memset(h2p[t][:, :, 1:8, 8], 0.0)

                # conv1: K=9, chunks of 2 samples (392 cols)
                for cot in range(2):
                    for cs in range(0, GE, 2):
                        ps = pmm.tile([128, 512], F32, tag="mm")
                        rhs = xpatch[:, cs:cs + 2, :]
                        nc.tensor.matmul(ps[:, :392],
                                         w1[:, cot, :],
                                         rhs, start=True, stop=True)
                        nc.scalar.activation(
                            h1p[cot][:, cs:cs + 2, 1:15, 1:15],
                            ps[:, :392], AF.Relu, bias=biasv[:, 0 + cot:1 + cot])

                # conv2: 16x16(pad) -> 7x7, chunks of 8 samples (392 cols)
                for cot in range(2):
                    for cs in range(0, GE, 8):
                        ps = pmm.tile([128, 512], F32, tag="mm")
                        kk = 0
                        for tap in range(9):
                            dy, dx = tap // 3, tap % 3
                            for cit in range(2):
                                rhs = h1p[cit][:, cs:cs + 8, dy:dy + 14:2, dx:dx + 14:2]
                                nc.tensor.matmul(
                                    ps[:, :392], wsl(w2, cit, tap, cot),
                                    rhs,
                                    start=(kk == 0), stop=(kk == 17))
                                kk += 1
                        nc.scalar.activation(
                            h2p[cot][:, cs:cs + 8, 1:8, 1:8],
                            ps[:, :392], AF.Relu, bias=biasv[:, 2 + cot:3 + cot])

                # conv3: 9x9(pad) -> 4x4, one chunk (GE*16 = 256 cols), no relu
                for cot in range(2):
                    ps = pmm.tile([128, 512], F32, tag="mm")
                    kk = 0
                    for tap in range(9):
                        dy, dx = tap // 3, tap % 3
                        for cit in range(2):
                            rhs = h2p[cit][:, :, dy:dy + 7:2, dx:dx + 7:2]
                            nc.tensor.matmul(
                                ps[:, :GE * 16], wsl(w3, cit, tap, cot),
                                rhs,
                                start=(kk == 0), stop=(kk == 17))
                            kk += 1
                    nc.scalar.activation(
                        zh[cot][:, n0:n0 + GE, :],
                        ps[:, :GE * 16], AF.Identity, bias=biasv[:, 4 + cot:5 + cot])

            # z_e output
            for h in range(2):
                nc.sync.dma_start(ze_d[:, h, :, :].transpose([1, 0, 2]), zh[h][:])

            enc_stack.close()

            # ---------------- VQ distance GEMM ----------------
            # score[n, code] = 2*z.e - |e|^2 + mask ; argmax == argmin d2
            vq_stack = ExitStack()
            vqp = vq_stack.enter_context(tc.tile_pool(name="vq", bufs=1))
            score = vqp.tile([64, S2K], F32)

            NCH = S2K // 512
            for ch in range(NCH):
                c0 = ch * 512
                maskt = vqp.tile([64, 512], F32, tag="mk", name=f"mk{ch}", bufs=2)
                nc.sync.dma_start(maskt[:], mask_d[:, c0:c0 + 512])
                ps = pvq.tile([64, 512], F32, tag="vq")
                for t in range(32):
                    ij, hf = t // 2, t % 2
                    ept = epp.tile([128, 512], F32, tag="ep", name=f"ep{ch}_{t}")
                    nc.sync.dma_start(ept[:], ep_d[t, :, c0:c0 + 512])
                    nc.tensor.matmul(ps, zh[hf][:, :, ij],
                                     ept[:],
                                     start=(t == 0), stop=(t == 31))
                nc.vector.tensor_tensor(score[:, c0:c0 + 512], ps,
                                        maskt[:], ALU.add)

            mx = zhp.tile([64, 8], F32)
            mi = zhp.tile([64, 8], U32)
            nc.vector.max(mx[:], score[:])
            nc.vector.max_index(mi[:], mx[:], score[:])

            # ---------------- gather E[idx] ----------------
            q = [zhp.tile([128, NS, 16], F32, tag=f"q{h}", name=f"q{h}") for h in range(2)]
            for n in range(NS):
                if no_gather:
                    for h in range(2):
                        nc.sync.dma_start(q[h][:, n, :], e4_d[n, h, :, :])
                else:
                    val = nc.values_load(mi[n:n + 1, 0:1],
                                         engines=(mybir.EngineType.Pool,),
                                         min_val=0, max_val=S2K - 1,
                                         skip_runtime_bounds_check=True)
                    for h in range(2):
                        nc.gpsimd.dma_start(q[h][:, n, :],
                                            e4_d[bass.ds(val, 1), h, :, :])
            for h in range(2):
                nc.sync.dma_start(zq_d[:, h, :, :].transpose([1, 0, 2]), q[h][:])

            vq_stack.close()

            # ---------------- decoder ----------------
            dec_stack = ExitStack()
            decp = dec_stack.enter_context(tc.tile_pool(name="dec", bufs=1))
            decp2 = dec_stack.enter_context(tc.tile_pool(name="dec2", bufs=1))
            wd = [wpool.tile([128, 4608], DT, tag="w", name=f"wdt{i}") for i in range(3)]
            for i in range(3):
                nc.gpsimd.dma_start(wd[i][:], wd_d[i][:])

            def upsample(src, dst, tab, H, W, pad, engs):
                """src [128, GD, H, W] -> dst interior [128, GD, 2H, 2W] with
                optional 1-border (dst preallocated; border zeroed by caller).
                engs: (e0, e1) engines for x / y passes."""
                x0t, wxt = tab
                Ho, Wo = 2 * H, 2 * W
                e0, e1 = engs
                tmp = decp2.tile([128, GD, H, Wo], DT, tag=f"tmp", name="tmp", bufs=2)
                dx = decp2.tile([128, GD, H, W - 1], DT, tag=f"dd", name="dx", bufs=2)
                e0.tensor_tensor(dx[:], src[:, :, :, 1:], src[:, :, :, :-1],
                                 ALU.subtract)
                for ox in range(Wo):
                    x0, w = int(x0t[ox]), float(wxt[ox])
                    if w == 0.0:
                        e0.tensor_copy(tmp[:, :, :, ox], src[:, :, :, x0])
                    else:
                        e0.tensor_scalar_mul(tmp[:, :, :, ox], dx[:, :, :, x0], w)
                        e0.tensor_tensor(tmp[:, :, :, ox], tmp[:, :, :, ox],
                                         src[:, :, :, x0], ALU.add)
                dy = decp2.tile([128, GD, H - 1, Wo], DT, tag=f"dd2", name="dy", bufs=2)
                e1.tensor_tensor(dy[:], tmp[:, :, 1:, :], tmp[:, :, :-1, :],
                                 ALU.subtract)
                p = 1 if pad else 0
                for oy in range(Ho):
                    y0, w = int(x0t[oy]), float(wxt[oy])
                    out = dst[:, :, oy + p, p:p + Wo]
                    if w == 0.0:
                        e1.tensor_copy(out, tmp[:, :, y0, :])
                    else:
                        e1.tensor_scalar_mul(out, dy[:, :, y0, :], w)
                        e1.tensor_tensor(out, out, tmp[:, :, y0, :], ALU.add)

            def zero_border(t, Hp, Wp):
                nc.gpsimd.memset(t[:, :, 0, :], 0.0)
                nc.gpsimd.memset(t[:, :, Hp - 1, :], 0.0)
                nc.gpsimd.memset(t[:, :, 1:Hp - 1, 0], 0.0)
                nc.gpsimd.memset(t[:, :, 1:Hp - 1, Wp - 1], 0.0)

            def dconv(win, src, cnt_list, outs, cg=4):
                """3x3 stride-1 valid conv on (padded) src tiles.
                Weight-stationary over chunk groups of cg: each (tap,cit)
                weight is loaded once per group and streamed over cg chunks.
                src: [2][128, GD, HS, WS]; cnt_list: (ns, ncnt, r0, Ho, Wo).
                outs(cot, chunk_idx, ps, ncols) -> epilogue."""
                for cot in range(2):
                    for g0 in range(0, len(cnt_list), cg):
                        grp = cnt_list[g0:g0 + cg]
                        pss = [pmm.tile([128, 512], F32, tag="mm",
                                        name=f"mm{cot}_{g0}_{i}")
                               for i in range(len(grp))]
                        kk = 0
                        for tap in range(9):
                            dyy, dxx = tap // 3, tap % 3
                            for cit in range(2):
                                w_ap = wsl(win, cit, tap, cot).bitcast(DT)
                                for i, (ns, ncnt, r0, Ho, Wo) in enumerate(grp):
                                    ncols = ncnt * Ho * Wo
                                    rhs = src[cit][:, ns:ns + ncnt,
                                                   r0 + dyy:r0 + dyy + Ho,
                                                   dxx:dxx + Wo]
                                    nc.tensor.matmul(
                                        pss[i][:, :ncols], w_ap,
                                        rhs.bitcast(DT),
                                        start=(kk == 0), stop=(kk == 17))
                                kk += 1
                        for i, (ns, ncnt, r0, Ho, Wo) in enumerate(grp):
                            outs(cot, g0 + i, pss[i], ncnt * Ho * Wo)

            for g in range(NGD):
                n0 = g * GD
                qb = [decp.tile([128, GD, 4, 4], DT, tag=f"qb{h}", name=f"qb{h}_{g}") for h in range(2)]
                for h in range(2):
                    nc.vector.tensor_copy(
                        qb[h][:], q[h][:, n0:n0 + GD, :])

                up1 = [decp.tile([128, GD, 10, 10], DT, tag=f"up1{h}", name=f"up1{h}_{g}") for h in range(2)]
                for h in range(2):
                    zero_border(up1[h], 10, 10)
                    upsample(qb[h], up1[h], ut1, 4, 4, True,
                             (nc.vector, nc.vector))

                h4 = [decp.tile([128, GD, 8, 8], DT, tag=f"h4{h}", name=f"h4{h}_{g}") for h in range(2)]

                def out_d0(cot, ci_, ps, ncols):
                    nc.scalar.activation(h4[cot][:],
                                         ps[:, :ncols], AF.Relu,
                                         bias=biasv[:, 6 + cot:7 + cot])
                dconv(wd[0], up1, [(0, GD, 0, 8, 8)], out_d0)

                up2 = [decp.tile([128, GD, 16, 16], DT, tag=f"up2{h}", name=f"up2{h}_{g}") for h in range(2)]
                for h in range(2):
                    upsample(h4[h], up2[h], ut2, 8, 8, False,
                             (nc.vector, nc.vector))

                h5 = [decp.tile([128, GD, 14, 14], DT, tag=f"h5{h}", name=f"h5{h}_{g}") for h in range(2)]

                def out_d1(cot, ci_, ps, ncols):
                    ns = ci_ * 2
                    nc.scalar.activation(
                        h5[cot][:, ns:ns + 2, :, :],
                        ps[:, :ncols], AF.Relu, bias=biasv[:, 8 + cot:9 + cot])
                dconv(wd[1], up2, [(i * 2, 2, 0, 14, 14) for i in range(GD // 2)],
                      out_d1)

                up3 = [decp.tile([128, GD, 30, 30], DT, tag=f"up3{h}", name=f"up3{h}_{g}") for h in range(2)]
                for h in range(2):
                    zero_border(up3[h], 30, 30)
                    upsample(h5[h], up3[h], ut3, 14, 14, True,
                             (nc.vector, nc.vector))

                h6 = [decp.tile([128, GD, 30, 30], DT, tag=f"h6{h}", name=f"h6{h}_{g}") for h in range(2)]
                for h in range(2):
                    zero_border(h6[h], 30, 30)

                def out_d2(cot, ci_, ps, ncols):
                    n, r = ci_ // 2, (ci_ % 2) * 14
                    nc.scalar.activation(
                        h6[cot][:, n, 1 + r:15 + r, 1:29],
                        ps[:, :ncols], AF.Relu, bias=biasv[:, 10 + cot:11 + cot])
                dconv(wd[2], up3, [(n, 1, r, 14, 28) for n in range(GD)
                                   for r in (0, 14)], out_d2)

                # output conv: co=1, col-packed 4-wide via tile_position.
                # 16 chunks per group: (sample, half-rows); 4 chunks share a
                # psum tile on col-groups 0/32/64/96.
                chunks = [(n, r) for n in range(GD) for r in (0, 14)]
                for s4 in range(0, len(chunks), 4):
                    ps = poc.tile([128, 512], F32, tag="oc")
                    pslist = ([poc.tile([128, 512], F32, tag="oc",
                                        name=f"oc{g}_{s4}_{j}") for j in range(4)]
                              if no_colpack else None)
                    for j in range(4):
                        n, r = chunks[s4 + j]
                        kk = 0
                        for tap in range(9):
                            dyy, dxx = tap // 3, tap % 3
                            for cit in range(2):
                                rhs = h6[cit][:, n, r + dyy:r + dyy + 14,
                                              dxx:dxx + 28]
                                if no_colpack:
                                    nc.tensor.matmul(
                                        pslist[j][0:1, :392],
                                        wo[:, cit * 9 + tap:cit * 9 + tap + 1].bitcast(DT),
                                        rhs.bitcast(DT),
                                        start=(kk == 0), stop=(kk == 17))
                                else:
                                    nc.tensor.matmul(
                                        ps[32 * j:32 * j + 1, :392],
                                        wo[:, cit * 9 + tap:cit * 9 + tap + 1].bitcast(DT),
                                        rhs.bitcast(DT),
                                        start=(kk == 0), stop=(kk == 17),
                                        tile_position=(0, 32 * j))
                                kk += 1
                    xt_sb = decp.tile([128, 392], F32, tag="xt", name=f"xt{g}_{s4}",
                                      bufs=4)
                    if no_colpack:
                        xts = [decp.tile([128, 392], F32, tag="xt",
                                         name=f"xts{g}_{s4}_{j}", bufs=4)
                               for j in range(4)]
                    for j in range(4):
                        n, r = chunks[s4 + j]
                        src_ps = pslist[j][0:1, :392] if no_colpack                             else ps[32 * j:32 * j + 1, :392]
                        dst_sb = xt_sb[0:1, :] if no_colpack                             else xt_sb[32 * j:32 * j + 1, :]
                        nc.scalar.activation(dst_sb, src_ps, AF.Sigmoid,
                                             bias=BO_IMM[0])
                        nc.sync.dma_start(
                            xt_d[n0 + n:n0 + n + 1, 28 * r:28 * r + 392],
                            dst_sb)
                if dbg and g == 0:
                    for nm, t in [("qb", qb[0]), ("up1", up1[0]), ("h4", h4[0]),
                                  ("up2", up2[0]), ("h5", h5[0]), ("up3", up3[0]),
                                  ("h6", h6[0])]:
                        nc.gpsimd.dma_start(dbg_d[nm][:], t[:])
            dec_stack.close()

    nc.compile()
    return nc


BO_IMM = [0.0]   # set by kernel() before build (bo is a runtime input scalar)


# ----------------------------------------------------------------------------
# kernel entry
# ----------------------------------------------------------------------------

def prepare(inputs):
    inputs = {k: np.ascontiguousarray(np.asarray(v)) for k, v in inputs.items()}
    x = inputs['x'].astype(np.float32)           # [512, 1, 28, 28]
    c = inputs['c'].astype(np.int64)             # [512]
    E = inputs['E'].astype(np.float32)           # [16384, 4096]
    BO_IMM[0] = float(np.asarray(inputs['bo']).reshape(-1)[0])

    counts = np.bincount(c, minlength=NCLS)
    order, S = _class_order(counts)
    S = max(S, 1)

    # sample ids in shard order: classes in `order`, stable within class
    ids_by_class = [np.nonzero(c == k)[0] for k in range(NCLS)]
    shard_ids = np.concatenate([ids_by_class[k] for k in order])
    # reference output permutation (stable sort by class ascending)
    perm = np.argsort(c, kind='stable')
    # row of each sample in the permuted outputs
    pos_in_perm = np.empty(NTOT, np.int64)
    pos_in_perm[perm] = np.arange(NTOT)

    # per-core classes + padded slice lists
    blocks = [shard_ids[64 * j:64 * j + 64] for j in range(NCORES)]
    slice_lists = []
    for blk in blocks:
        cls = list(dict.fromkeys(c[blk].tolist()))   # preserves order
        while len(cls) < S:
            cls.append(cls[-1])
        assert len(cls) == S, f"block spans {len(cls)} > S={S} classes"
        slice_lists.append(cls)

    # ---- weights (shared across cores) ----
    w1 = np.ascontiguousarray(
        inputs['We0'].reshape(2, 128, 3, 3).transpose(2, 3, 0, 1).reshape(9, 2, 128),
        np.float32)
    w2 = _pack_w(inputs['We1'])
    w3 = _pack_w(inputs['We2'])
    wd = [_pack_w(inputs[f'Wd{i}']) for i in range(3)]
    wo = np.ascontiguousarray(
        inputs['Wo'].reshape(2, 128, 3, 3).transpose(1, 0, 2, 3).reshape(128, 18),
        np.float32)
    biasv = np.zeros((128, 12), np.float32)
    for L, bn in enumerate(['be0', 'be1', 'be2', 'bd0', 'bd1', 'bd2']):
        b = inputs[bn].astype(np.float32)
        biasv[:, 2 * L] = b[:128]
        biasv[:, 2 * L + 1] = b[128:]

    # ---- per-core tensors ----
    in_maps = []
    for j in range(NCORES):
        blk = blocks[j]
        cls = slice_lists[j]
        xb = x[blk, 0]                                   # [64, 28, 28]
        xp = np.pad(xb, ((0, 0), (1, 1), (1, 1)))
        patches = np.empty((9, 64, 196), np.float32)
        for t in range(9):
            dy, dx = t // 3, t % 3
            patches[t] = xp[:, dy:dy + 28:2, dx:dx + 28:2].reshape(64, 196)

        E_cat = np.concatenate([E[k * K:(k + 1) * K] for k in cls], 0)  # [S2K, 4096]
        arr = (2.0 * E_cat).reshape(S * K, 2, 128, 16)
        epermT = np.ascontiguousarray(
            arr.transpose(3, 1, 2, 0).reshape(32, 128, S * K), np.float32)
        esqneg = -(E_cat ** 2).sum(1, dtype=np.float32)
        maskadd = np.full((64, S * K), NEG, np.float32)
        for i, sid in enumerate(blk):
            s = cls.index(int(c[sid]))
            maskadd[i, s * K:(s + 1) * K] = esqneg[s * K:(s + 1) * K]
        e4 = np.ascontiguousarray(E_cat.reshape(S * K, 2, 128, 16), np.float32)

        in_maps.append(dict(
            xpatch=patches, w1=w1, w2=w2, w3=w3,
            wd0=wd[0], wd1=wd[1], wd2=wd[2], wo=wo, biasv=biasv,
            epermT=epermT, maskadd=maskadd, e4=e4,
        ))

    return in_maps, blocks, pos_in_perm, S


def assemble(results, blocks, pos_in_perm):
    # ---- reassemble ----
    x_tilde = np.empty((NTOT, 1, 28, 28), np.float32)
    z_e = np.empty((NTOT, D, 1, 1), np.float32)
    z_q = np.empty((NTOT, D, 1, 1), np.float32)
    for j in range(len(blocks)):
        blk = blocks[j]
        r = results[j]
        xt = r['xt'].reshape(64, 28, 28)
        ze = r['ze'].reshape(64, D)       # [n, (half,co,ij)] == [n, d]
        zq = r['zq'].reshape(64, D)
        for i, sid in enumerate(blk):
            z_e[sid, :, 0, 0] = ze[i]
            p = pos_in_perm[sid]
            x_tilde[p, 0] = xt[i]
            z_q[p, :, 0, 0] = zq[i]
    return (x_tilde, z_e, z_q)


def kernel(**inputs):
    in_maps, blocks, pos_in_perm, S = prepare(inputs)
    nc = build_graph(S, dec_bf16=True)
    res = run_bass_kernel_spmd(nc, in_maps, core_ids=list(range(NCORES)))
    return assemble(res.results, blocks, pos_in_perm)
